# revision 1
# baseline (speedup 1.0000x reference)
"""Trainium2 Bass kernel for nn_MiniMHCLM (moe_routing).

Strategy (8 NeuronCores, SPMD, no collectives):
  - vocab-sharded head matmul: core i holds w_head rows [i*VS:(i+1)*VS]
    (host-sliced, zero-padded to uniform VS) and computes logits for all
    4096 tokens x its vocab slice.  Host concatenates along vocab.
  - the cheap per-token pipeline (embed gather, RMS+phi coeffs, Sinkhorn,
    gather/scatter mixing) is replicated on every core.
  - embedding table is staged host-side in bf16 (numerically identical to
    the reference's `embed[ids].astype(bf16)`), gathered on-device via
    indirect DMA.
  - w_head slice is converted to bf16 and PE-transposed into a k-major
    SBUF-resident tile at kernel start.
  - head matmul in bf16 with fp32 PSUM accumulation; PSUM evacuated by
    ACT/DVE copies (DMA cannot read PSUM) and DMA'd to DRAM fp32.
  - super-chunks of g 128-token chunks are software-pipelined: pass C
    (mixing + head matmul, PE-heavy) of super-chunk sc overlaps pass A
    (gather + coeffs, ACT/DVE/DMA-heavy) of super-chunk sc+1.
"""

import numpy as np

HC, C, TMAX = 4, 256, 8
RMS_EPS, PRE_EPS, SINK_EPS, POST_MULT = 1e-6, 1e-4, 1e-6, 2.0
VOCAB = 50257
B, S = 2, 2048
K = HC * C            # 1024
M = HC * HC + 2 * HC  # 24
NKC = K // 128        # 8 k-chunks
NCORES = 8


class _Cfg:
    def __init__(self, vocab, vs, vpad, nt, g, nsc, scs=None):
        self.vocab = vocab      # rows in the (full) embedding table
        self.vs = vs            # vocab-slice rows per core (uniform, padded)
        self.vpad = vpad        # vs padded for transpose/head loop
        self.nt = nt            # total tokens
        self.g = g              # max 128-token chunks per super-chunk
        self.scs = scs if scs is not None else [g] * nsc
        assert max(self.scs) <= g and sum(self.scs) == nt // 128
        assert all(a + b <= 2 * g for a, b in zip(self.scs, self.scs[1:]))
        self.vw = min(512, vpad)
        assert vpad % self.vw == 0 and vpad % 16 == 0
        self.nv = vpad // self.vw


REAL = _Cfg(vocab=VOCAB, vs=6283, vpad=6656, nt=B * S, g=8, nsc=None,
            scs=[2, 2, 4, 8, 8, 8])


def _build(cfg: _Cfg, gather_mode="indirect", head_nv=None, tiny_out=False,
           sink_iters=None):
    from contextlib import ExitStack
    from concourse import bass, bacc, mybir
    import concourse.tile as tile
    from concourse.masks import make_identity

    f32 = mybir.dt.float32
    bf16 = mybir.dt.bfloat16
    i32 = mybir.dt.int32
    AX = mybir.AxisListType
    OP = mybir.AluOpType
    AF = mybir.ActivationFunctionType

    nt, g, vs, vpad, vw, nv = (
        cfg.nt, cfg.g, cfg.vs, cfg.vpad, cfg.vw, cfg.nv)
    scs = cfg.scs
    n_sc = len(scs)
    sc_off = [sum(scs[:i]) for i in range(n_sc)]
    nchunks = nt // 128
    g4, g16 = g * 4, g * 16
    n_head_v = nv if head_nv is None else head_nv

    nc = bacc.Bacc(target_bir_lowering=False)
    ids_p = nc.declare_dram_parameter("ids", [128, nchunks], i32, False)
    emb_p = nc.declare_dram_parameter("emb", [cfg.vocab, K], bf16, False)
    wvt_p = nc.declare_dram_parameter("wvt", [K, vpad], bf16, False)
    wit_p = nc.declare_dram_parameter("wit", [C, C], bf16, False)
    phi_p = nc.declare_dram_parameter("phi", [K, M], bf16, False)
    b_p = nc.declare_dram_parameter("b", [1, M], f32, False)
    al_p = nc.declare_dram_parameter("al", [1, 3], f32, False)
    out_p = nc.declare_dram_parameter(
        "out", [128, vw] if tiny_out else [nt, vs], f32, True)

    with ExitStack() as ctx:
        tc = ctx.enter_context(tile.TileContext(nc))
        const = ctx.enter_context(tc.tile_pool(name="const", bufs=1))
        wtp = ctx.enter_context(tc.tile_pool(name="wtp", bufs=1))
        xbfp = ctx.enter_context(tc.tile_pool(name="xbfp", bufs=1))
        scp = ctx.enter_context(tc.tile_pool(name="scp", bufs=2))
        wkA = ctx.enter_context(tc.tile_pool(name="wkA", bufs=2))
        wkB = ctx.enter_context(tc.tile_pool(name="wkB", bufs=2))
        wkC = ctx.enter_context(tc.tile_pool(name="wkC", bufs=3))
        wk3 = ctx.enter_context(tc.tile_pool(name="wk3", bufs=6))
        outp = ctx.enter_context(tc.tile_pool(name="outp", bufs=6))
        pst = ctx.enter_context(tc.tile_pool(name="pst", bufs=2, space="PSUM"))
        psc = ctx.enter_context(tc.tile_pool(name="psc", bufs=1, space="PSUM"))
        pss = ctx.enter_context(tc.tile_pool(name="pss", bufs=2, space="PSUM"))
        psh = ctx.enter_context(tc.tile_pool(name="psh", bufs=3, space="PSUM"))

        # ---------------- prep ----------------
        ident = const.tile([128, 128], bf16)
        make_identity(nc, ident[:])

        cst = const.tile([128, 2], f32)
        nc.vector.memset(cst[:, 0:1], 0.0)
        nc.vector.memset(cst[:, 1:2], RMS_EPS)
        zero_b = cst[:, 0:1]
        eps_b = cst[:, 1:2]

        phi_sb = const.tile([128, NKC * M], bf16)
        for kc in range(NKC):
            nc.sync.dma_start(out=phi_sb[:, kc * M:(kc + 1) * M],
                              in_=phi_p[kc * 128:(kc + 1) * 128, :])
        # broadcast b/alpha row to all 128 partitions via stride-0 DMA reads
        b_bc = const.tile([128, M], f32)
        nc.sync.dma_start(out=b_bc[:], in_=b_p[0:1, :].to_broadcast([128, M]))
        al_bc = const.tile([128, 3], f32)
        nc.sync.dma_start(out=al_bc[:], in_=al_p[0:1, :].to_broadcast([128, 3]))

        ids_all = const.tile([128, nchunks], i32)
        nc.sync.dma_start(out=ids_all[:], in_=ids_p[:, :])

        # w_inner^T (host-pretransposed bf16, k-major): one strided DMA
        w_iT = const.tile([128, 2 * C], bf16)
        nc.sync.dma_start(
            out=w_iT[:].rearrange("p (kc o) -> p kc o", kc=2),
            in_=wit_p[:, :].rearrange("(kc p) o -> p kc o", p=128))

        # w_head^T slice (host-pretransposed bf16): 8 row-band DMAs
        wt_all = wtp.tile([128, NKC * vpad], bf16, tag="wt_all")
        wt3 = wt_all[:].rearrange("p (kc v) -> p kc v", kc=NKC)
        for kc in range(NKC):
            nc.sync.dma_start(out=wt_all[:, kc * vpad:(kc + 1) * vpad],
                              in_=wvt_p[kc * 128:(kc + 1) * 128, :])

        # ---------------- pipelined main ----------------
        sc_state = {}

        def pass_a_chunk(sc, c):
            cc = sc_off[sc] + c
            if sc not in sc_state:
                sc_state[sc] = dict(
                    m_all=scp.tile([128, g16], f32, tag="m_all", name=f"m_all{sc}"),
                    hpre=scp.tile([128, g4], f32, tag="hpre", name=f"hpre{sc}"),
                    hpost2=scp.tile([128, g4], f32, tag="hpost2", name=f"hpost2{sc}"),
                    scl=scp.tile([128, g], f32, tag="scl", name=f"scl{sc}"),
                    sqs=scp.tile([128, g], f32, tag="sqs", name=f"sqs{sc}"),
                    lg=scp.tile([128, g * M], f32, tag="lg", name=f"lg{sc}"),
                    xbs={},
                )
            st = sc_state[sc]
            xb = xbfp.tile([128, K], bf16, tag=f"xb{cc % (2 * g)}",
                           name=f"xb{cc}")
            if gather_mode == "indirect":
                nc.gpsimd.indirect_dma_start(
                    out=xb[:], out_offset=None,
                    in_=emb_p[:, :],
                    in_offset=bass.IndirectOffsetOnAxis(
                        ap=ids_all[:, cc:cc + 1], axis=0))
            else:
                nc.sync.dma_start(out=xb[:],
                                  in_=emb_p[cc * 128:(cc + 1) * 128, :])
            st["xbs"][c] = xb

            dump = wkA.tile([128, K], bf16, tag="dump")
            nc.scalar.activation(out=dump[:], in_=xb[:], func=AF.Square,
                                 bias=zero_b, accum_out=st["sqs"][:, c:c + 1])

            ptx = pst.tile([128, 1024], bf16, tag="pst")
            for kc in range(NKC):
                nc.tensor.transpose(
                    out=ptx[:, kc * 128:(kc + 1) * 128],
                    in_=xb[:, kc * 128:(kc + 1) * 128], identity=ident[:])
            xT = wkA.tile([128, 1024], bf16, tag="xT")
            nc.scalar.copy(xT[:], ptx[:])

            pc = psc.tile([128, C], f32, tag="psc")
            for kc in range(NKC):
                nc.tensor.matmul(
                    out=pc[:, :M],
                    lhsT=xT[:, kc * 128:(kc + 1) * 128],
                    rhs=phi_sb[:, kc * M:(kc + 1) * M],
                    start=(kc == 0), stop=(kc == NKC - 1))
            nc.vector.tensor_copy(st["lg"][:, c * M:(c + 1) * M], pc[:, :M])

        def pass_ab_post(sc):
            g_sc = scs[sc]
            st = sc_state[sc]
            m_all, hpre, hpost2 = st["m_all"], st["hpre"], st["hpost2"]
            scl, sqs, lg = st["scl"], st["sqs"], st["lg"]
            # rms scales for the whole super-chunk
            nc.scalar.activation(out=scl[:, :g_sc], in_=sqs[:, :g_sc],
                                 func=AF.Sqrt, scale=1.0 / K, bias=eps_b)
            nc.vector.reciprocal(scl[:, :g_sc], scl[:, :g_sc])
            lgv = lg[:, :g_sc * M].rearrange("p (c m) -> p c m", m=M)
            for c in range(g_sc):
                nc.vector.tensor_scalar_mul(
                    lg[:, c * M:(c + 1) * M], lg[:, c * M:(c + 1) * M],
                    scl[:, c:c + 1])
            nc.vector.tensor_tensor(
                out=lgv, in0=lgv,
                in1=b_bc[:][:, None, :].to_broadcast([128, g_sc, M]), op=OP.add)
            # coeff activations, batched over the super-chunk
            nc.scalar.activation(
                out=hpre[:, :g_sc * 4].rearrange("p (c f) -> p c f", f=4),
                in_=lgv[:, :, 0:4], func=AF.Sigmoid, bias=zero_b,
                scale=al_bc[:, 0:1])
            nc.vector.tensor_scalar_add(hpre[:, :g_sc * 4],
                                        hpre[:, :g_sc * 4], PRE_EPS)
            nc.scalar.activation(
                out=hpost2[:, :g_sc * 4].rearrange("p (c f) -> p c f", f=4),
                in_=lgv[:, :, 4:8], func=AF.Sigmoid, bias=zero_b,
                scale=al_bc[:, 1:2])
            nc.vector.tensor_scalar_mul(hpost2[:, :g_sc * 4],
                                        hpost2[:, :g_sc * 4], POST_MULT)
            nc.scalar.activation(
                out=m_all[:, :g_sc * 16].rearrange("p (c f) -> p c f", f=16),
                in_=lgv[:, :, 8:24], func=AF.Exp, bias=zero_b,
                scale=al_bc[:, 2:3])

            # batched Sinkhorn
            ma = m_all[:, :g_sc * 16]
            mv3 = ma.rearrange("p (a i) -> p a i", i=4)
            mv4 = ma.rearrange("p (c o i) -> p c o i", o=4, i=4)
            mv4t = ma.rearrange("p (c o i) -> p c i o", o=4, i=4)
            for _ in range(TMAX if sink_iters is None else sink_iters):
                rs = wkB.tile([128, g4], f32, tag="rs")
                rsv = rs[:, :g_sc * 4]
                nc.vector.tensor_reduce(rsv, mv3, axis=AX.X, op=OP.add)
                nc.vector.tensor_scalar_add(rsv, rsv, SINK_EPS)
                nc.vector.reciprocal(rsv, rsv)
                nc.vector.tensor_tensor(
                    out=mv3, in0=mv3,
                    in1=rsv[:, :, None].to_broadcast([128, g_sc * 4, 4]),
                    op=OP.mult)
                cs = wkB.tile([128, g4], f32, tag="cs")
                csv = cs[:, :g_sc * 4]
                nc.vector.tensor_reduce(csv, mv4t, axis=AX.X, op=OP.add)
                nc.vector.tensor_scalar_add(csv, csv, SINK_EPS)
                nc.vector.reciprocal(csv, csv)
                nc.vector.tensor_tensor(
                    out=mv4, in0=mv4,
                    in1=csv.rearrange("p (c i) -> p c i", i=4)
                         [:, :, None, :].to_broadcast([128, g_sc, 4, 4]),
                    op=OP.mult)

        mix_out = {}

        def pass_mix_chunk(sc, c):
            st = sc_state[sc]
            xb = st["xbs"][c]
            m_all, hpre, hpost2 = st["m_all"], st["hpre"], st["hpost2"]

            # x_in = sum_i h_pre[i] * x[i]
            xin = wkC.tile([128, C], bf16, tag="xin")
            nc.vector.tensor_scalar_mul(
                xin[:], xb[:, 0:C], hpre[:, c * 4:c * 4 + 1])
            for i in range(1, HC):
                tmp = wk3.tile([128, C], bf16, tag="tmp")
                nc.vector.tensor_scalar_mul(
                    tmp[:], xb[:, i * C:(i + 1) * C],
                    hpre[:, c * 4 + i:c * 4 + i + 1])
                eng = nc.vector if i % 2 else nc.gpsimd
                eng.tensor_add(xin[:], xin[:], tmp[:])
            # x_in^T
            pti = pss.tile([128, C], bf16, tag="pss")
            for ib in range(2):
                nc.tensor.transpose(
                    out=pti[:, ib * 128:(ib + 1) * 128],
                    in_=xin[:, ib * 128:(ib + 1) * 128], identity=ident[:])
            xiT = wkC.tile([128, C], bf16, tag="xiT")
            nc.scalar.copy(xiT[:], pti[:, :C])
            # f_out = x_in @ w_inner.T
            pf = pss.tile([128, C], f32, tag="pss")
            for ib in range(2):
                nc.tensor.matmul(
                    out=pf[:], lhsT=xiT[:, ib * 128:(ib + 1) * 128],
                    rhs=w_iT[:, ib * C:(ib + 1) * C],
                    start=(ib == 0), stop=(ib == 1))
            fo = wkC.tile([128, C], bf16, tag="fo")
            nc.scalar.copy(fo[:], pf[:])

            # x_merge[o] = sum_i h_res[o,i]*x[i] + h_post2[o]*f_out
            xmg = wkC.tile([128, K], bf16, tag="xmg")
            for o in range(HC):
                seg = xmg[:, o * C:(o + 1) * C]
                base = c * 16 + o * 4
                nc.vector.tensor_scalar_mul(
                    seg, xb[:, 0:C], m_all[:, base:base + 1])
                for i in range(1, HC):
                    tmp = wk3.tile([128, C], bf16, tag="tmp")
                    nc.vector.tensor_scalar_mul(
                        tmp[:], xb[:, i * C:(i + 1) * C],
                        m_all[:, base + i:base + i + 1])
                    eng = nc.vector if i % 2 else nc.gpsimd
                    eng.tensor_add(seg, seg, tmp[:])
                tmp = wk3.tile([128, C], bf16, tag="tmp")
                nc.vector.tensor_scalar_mul(
                    tmp[:], fo[:], hpost2[:, c * 4 + o:c * 4 + o + 1])
                nc.gpsimd.tensor_add(seg, seg, tmp[:])

            # x_merge^T (k-major)
            ptm = pst.tile([128, 1024], bf16, tag="pst")
            for kc in range(NKC):
                nc.tensor.transpose(
                    out=ptm[:, kc * 128:(kc + 1) * 128],
                    in_=xmg[:, kc * 128:(kc + 1) * 128], identity=ident[:])
            xmT = wkC.tile([128, 1024], bf16, tag="xmT")
            nc.scalar.copy(xmT[:], ptm[:])
            mix_out[(sc, c)] = xmT
            # drop our reference so the xb slot can be reused next SC
            del st["xbs"][c]

        def pass_head_chunk(sc, c):
            cc = sc_off[sc] + c
            t0 = cc * 128
            xmT = mix_out.pop((sc, c))
            for v in range(n_head_v):
                ph = psh.tile([128, vw], f32, tag="psh")
                for kc in range(NKC):
                    nc.tensor.matmul(
                        out=ph[:],
                        lhsT=xmT[:, kc * 128:(kc + 1) * 128],
                        rhs=wt3[:, kc, v * vw:(v + 1) * vw],
                        start=(kc == 0), stop=(kc == NKC - 1))
                stg = outp.tile([128, vw], f32, tag="stg")
                nc.scalar.copy(stg[:], ph[:])
                w = min(vw, vs - v * vw)
                deng = nc.sync if v % 2 == 0 else nc.gpsimd
                if tiny_out:
                    if sc == 0 and c == 0 and v == 0:
                        nc.sync.dma_start(out=out_p[:, :], in_=stg[:, :])
                elif w > 0:
                    deng.dma_start(
                        out=out_p[t0:t0 + 128, v * vw:v * vw + w],
                        in_=stg[:, :w])

        # emission: software pipeline over super-chunks; mix(c+1) and
        # pass-A(sc+1) are emitted BEFORE head(c) so the scheduler's
        # priority order lets them interleave into the head-MM burst.
        for c in range(scs[0]):
            pass_a_chunk(0, c)
        pass_ab_post(0)
        for sc in range(n_sc):
            g_sc = scs[sc]
            nxt = scs[sc + 1] if sc + 1 < n_sc else 0
            a_done = 0
            pass_mix_chunk(sc, 0)
            for c in range(g_sc):
                if c + 1 < g_sc:
                    pass_mix_chunk(sc, c + 1)
                target = min(nxt, 2 * (c + 1))
                while a_done < target:
                    pass_a_chunk(sc + 1, a_done)
                    a_done += 1
                    if a_done == nxt:
                        pass_ab_post(sc + 1)
                pass_head_chunk(sc, c)
            del sc_state[sc]

    if not nc.is_finalized():
        nc.finalize()
    return nc


_NC_CACHE = {}


def _get_nc(cfg, **kw):
    key = (cfg.vocab, cfg.vs, cfg.vpad, cfg.nt, tuple(cfg.scs),
           tuple(sorted(kw.items())))
    if key not in _NC_CACHE:
        _NC_CACHE[key] = _build(cfg, **kw)
    return _NC_CACHE[key]


def _make_in_maps(cfg, input_ids, embed, w_inner, w_head, phi, b,
                  alpha_pre, alpha_post, alpha_res):
    import ml_dtypes
    bf = ml_dtypes.bfloat16
    nt, vs = cfg.nt, cfg.vs

    ids = np.ascontiguousarray(
        np.asarray(input_ids).astype(np.int32).reshape(-1, 128).T)  # [128, nchunks]
    emb = np.asarray(embed).astype(bf)
    phi_np = np.asarray(phi).astype(bf)
    wit = np.ascontiguousarray(np.asarray(w_inner).astype(bf).T)
    b_np = np.asarray(b, dtype=np.float32).reshape(1, M)
    al = np.array([[np.asarray(alpha_pre).reshape(-1)[0],
                    np.asarray(alpha_post).reshape(-1)[0],
                    np.asarray(alpha_res).reshape(-1)[0]]], dtype=np.float32)
    wh_t = np.asarray(w_head).astype(bf).T            # [K, vocab]

    in_maps = []
    for i in range(NCORES):
        sl = wh_t[:, i * vs:(i + 1) * vs]
        wvt = np.zeros((K, cfg.vpad), bf)
        wvt[:, :sl.shape[1]] = sl
        in_maps.append(dict(ids=ids, emb=emb, wvt=wvt,
                            wit=wit, phi=phi_np, b=b_np, al=al))
    return in_maps


def _run(cfg, in_maps, trace=False):
    from concourse.bass_utils import run_bass_kernel_spmd
    nc = _get_nc(cfg)
    return run_bass_kernel_spmd(nc, in_maps, list(range(NCORES)), trace=trace)


def kernel(input_ids, embed, w_inner, w_head, phi, b,
           alpha_pre, alpha_post, alpha_res):
    cfg = REAL
    in_maps = _make_in_maps(cfg, input_ids, embed, w_inner, w_head, phi, b,
                            alpha_pre, alpha_post, alpha_res)
    res = _run(cfg, in_maps).results
    out = np.concatenate([np.asarray(res[i]["out"]) for i in range(NCORES)],
                         axis=1)[:, :VOCAB]
    return np.ascontiguousarray(out.reshape(B, S, VOCAB).astype(np.float32))

